# revision 15
# baseline (speedup 1.0000x reference)
"""Multi-head attention (B=4, S=2048, D=512, H=8) for 8 Trainium2 NeuronCores.

Sharding: batch (4) x head-group (2 groups of 4 heads) -> 8 cores.
Each core computes, for its (batch b, head-group hg):
  - q/k projections in [d_head, seq] layout (f32r matmuls, TF32-like precision)
  - v projection in [seq, d_head] layout
  - pass A per head: scores = q_i . k_j for the causal-active blocks only,
    fused exp+rowsum on the scalar engine, normalize on vector engine,
    DMA only active blocks of the attention output (upper triangle is
    never written; output buffers are zero-initialized by the runtime)
  - pass B per head-pair: transposed scores sT[j,i], exp, ctx = attn @ v
    accumulated on PE; normalized by a PE-broadcast reciprocal row
  - output projection written transposed; host sums the two head-group
    partials, transposes, and adds b_o.

Outputs: (x [4,2048,512] f32, attention [4,8,2048,2048] f32), matching the
reference `(x, attention)` tuple.
"""

import sys

sys.path.insert(0, "/opt/trn_rl_repo")
sys.path.insert(0, "/root/.axon_site/_ro/trn_rl_repo")

import math
from contextlib import ExitStack

import numpy as np

B, S, D, H, DH = 4, 2048, 512, 8, 64
HPC = 4  # heads per core
SC = 1.0 / math.sqrt(DH)  # 0.125
NEG = -1.0e30
NT = S // 128  # 16 q-tiles / j-blocks
NC4 = D // 128  # 4 k-tiles of the model dim
ICW = 512  # i-chunk width in pass B
NIC = S // ICW  # 4


def _build_nc():
    import concourse.tile as tile
    from concourse import bacc, mybir

    f32 = mybir.dt.float32
    f32r = mybir.dt.float32r
    bf16 = mybir.dt.bfloat16
    Exp = mybir.ActivationFunctionType.Exp

    nc = bacc.Bacc("TRN2", target_bir_lowering=False, debug=False, num_devices=8)

    # ---- DRAM I/O (per core) ----
    d_qt = nc.dram_tensor("qt", [D, S], f32r, kind="ExternalInput")
    d_kt = nc.dram_tensor("kt", [D, S], f32r, kind="ExternalInput")
    d_vt = nc.dram_tensor("vt", [D, S], f32r, kind="ExternalInput")
    d_wq = nc.dram_tensor("wq", [D, 256], f32r, kind="ExternalInput")
    d_wk = nc.dram_tensor("wk", [D, 256], f32r, kind="ExternalInput")
    d_wv = nc.dram_tensor("wv", [D, 256], f32r, kind="ExternalInput")
    d_wo = nc.dram_tensor("wo", [64, HPC, D], f32r, kind="ExternalInput")
    d_bq = nc.dram_tensor("bq", [256, 1], f32, kind="ExternalInput")
    d_bk = nc.dram_tensor("bk", [256, 1], f32, kind="ExternalInput")
    d_bv = nc.dram_tensor("bv", [1, 256], f32r, kind="ExternalInput")
    d_tri = nc.dram_tensor("tri", [128, 128], f32, kind="ExternalInput")
    d_trit = nc.dram_tensor("trit", [128, 128], f32, kind="ExternalInput")
    d_id = nc.dram_tensor("ident", [128, 128], f32, kind="ExternalInput")
    d_ones = nc.dram_tensor("ones", [1, 128], f32r, kind="ExternalInput")
    d_attn = nc.dram_tensor("attn", [HPC, S, S], f32, kind="ExternalOutput")
    d_xt = nc.dram_tensor("xt", [D, S], f32, kind="ExternalOutput")

    with tile.TileContext(nc) as tc, ExitStack() as ctx:
        sbp = ctx.enter_context(tc.tile_pool(name="sbp", bufs=1))
        sbw = ctx.enter_context(tc.tile_pool(name="sbw", bufs=3))
        sbr = ctx.enter_context(tc.tile_pool(name="sbr", bufs=4))
        inb = ctx.enter_context(tc.tile_pool(name="inb", bufs=1))
        psS = ctx.enter_context(tc.tile_pool(name="psS", bufs=2, space="PSUM"))
        psC = ctx.enter_context(tc.tile_pool(name="psC", bufs=2, space="PSUM"))
        dsc = ctx.enter_context(tc.tile_pool(name="dsc", bufs=2, space="DRAM"))

        # ---- constants / weights to SBUF ----
        tri = sbp.tile([128, 128], f32)
        nc.sync.dma_start(tri[:], d_tri.ap())
        trit = sbp.tile([128, 128], f32)
        nc.sync.dma_start(trit[:], d_trit.ap())
        ident = sbp.tile([128, 128], f32)
        nc.sync.dma_start(ident[:], d_id.ap())
        ones = sbp.tile([1, 128], f32r)
        nc.sync.dma_start(ones[:], d_ones.ap())
        wq = sbp.tile([128, NC4, 256], f32r)
        nc.sync.dma_start(wq[:], d_wq.ap().rearrange("(kt p) c -> p kt c", p=128))
        wk = sbp.tile([128, NC4, 256], f32r)
        nc.sync.dma_start(wk[:], d_wk.ap().rearrange("(kt p) c -> p kt c", p=128))
        wv = sbp.tile([128, NC4, 256], f32r)
        nc.sync.dma_start(wv[:], d_wv.ap().rearrange("(kt p) c -> p kt c", p=128))
        wo = sbp.tile([64, HPC, D], f32r)
        nc.sync.dma_start(wo[:], d_wo.ap())
        bq = sbp.tile([128, 2], f32)
        nc.sync.dma_start(bq[:], d_bq.ap().rearrange("(kt p) one -> p (kt one)", p=128))
        bk = sbp.tile([128, 2], f32)
        nc.sync.dma_start(bk[:], d_bk.ap().rearrange("(kt p) one -> p (kt one)", p=128))
        bvr = sbp.tile([1, 256], f32r)
        nc.sync.dma_start(bvr[:], d_bv.ap())

        # b_v broadcast to 128 partitions via K=1 outer product
        psum_bvb = psS.tile([128, 1024], f32, tag="big")
        nc.tensor.matmul(
            psum_bvb[:, 0:256], ones[:], bvr[:], start=True, stop=True
        )
        bvb = sbp.tile([128, 256], f32)
        nc.vector.tensor_copy(bvb[:], psum_bvb[:, 0:256])

        # ---- persistent activations ----
        # qT/kT: [ch 256, seq] as 2 tiles of 128 partitions (ch = head*64+dh)
        qT = [sbp.tile([128, S], bf16, tag=f"qT{i}", name=f"qT{i}") for i in range(2)]
        kT = [sbp.tile([128, S], bf16, tag=f"kT{i}", name=f"kT{i}") for i in range(2)]
        # v natural: [j 128, jblock 16, ch 256]
        vna = sbp.tile([128, NT, 256], bf16)
        # ctxT: per head [64, seq] at partition base 0 (c-subtile h)
        ctxT = [sbp.tile([64, S], f32r, tag=f"ctxT{i}", name=f"ctxT{i}") for i in range(HPC)]

        # ---- projections ----
        for sc4 in range(NIC):
            s0 = sc4 * 512
            qtc = inb.tile([128, NC4, 512], f32r, tag="qtc")
            nc.sync.dma_start(
                qtc[:], d_qt.ap().rearrange("(kt p) s -> p kt s", p=128)[:, :, s0 : s0 + 512]
            )
            ktc = inb.tile([128, NC4, 512], f32r, tag="ktc")
            nc.sync.dma_start(
                ktc[:], d_kt.ap().rearrange("(kt p) s -> p kt s", p=128)[:, :, s0 : s0 + 512]
            )
            for kh in range(2):
                pq = psS.tile([128, 1024], f32, tag="big")
                pk = psS.tile([128, 1024], f32, tag="big")
                for k4 in range(NC4):
                    nc.tensor.matmul(
                        pq[:, 0:512],
                        wq[:, k4, kh * 128 : kh * 128 + 128],
                        qtc[:, k4, :],
                        start=(k4 == 0),
                        stop=(k4 == NC4 - 1),
                    )
                for k4 in range(NC4):
                    nc.tensor.matmul(
                        pk[:, 0:512],
                        wk[:, k4, kh * 128 : kh * 128 + 128],
                        ktc[:, k4, :],
                        start=(k4 == 0),
                        stop=(k4 == NC4 - 1),
                    )
                nc.vector.tensor_scalar_add(
                    qT[kh][:, s0 : s0 + 512], pq[:, 0:512], bq[:, kh : kh + 1]
                )
                nc.vector.tensor_scalar_add(
                    kT[kh][:, s0 : s0 + 512], pk[:, 0:512], bk[:, kh : kh + 1]
                )
        for sc4 in range(NIC):
            s0 = sc4 * 512
            vtc = inb.tile([128, NC4, 512], f32r, tag="vtc")
            nc.sync.dma_start(
                vtc[:], d_vt.ap().rearrange("(kt p) s -> p kt s", p=128)[:, :, s0 : s0 + 512]
            )
            for jbl in range(4):  # 4 j-blocks of 128 within this 512 chunk
                jb = sc4 * 4 + jbl
                pv = psS.tile([128, 1024], f32, tag="big")
                for k4 in range(NC4):
                    nc.tensor.matmul(
                        pv[:, 0:256],
                        vtc[:, k4, jbl * 128 : jbl * 128 + 128],
                        wv[:, k4, :],
                        start=(k4 == 0),
                        stop=(k4 == NC4 - 1),
                    )
                nc.vector.tensor_add(vna[:, jb, :], pv[:, 0:256], bvb[:])

        attn_ap = d_attn.ap()

        # ---- attention, head pairs ----
        for g in range(2):
            recips = []
            for par in range(2):
                h = 2 * g + par
                rbase = par * 64
                recip_all = sbr.tile([128, NT], f32, tag="recip")
                recips.append(recip_all)
                # ----- pass A -----
                for t in range(NT):
                    W = (t + 1) * 128
                    expA = sbw.tile([128, S], f32, tag="expA")
                    lsum = sbr.tile([128, 2], f32, tag="lsum")
                    nchunks = (W + 1023) // 1024
                    for cidx in range(nchunks):
                        cb = cidx * 1024
                        wc = min(1024, W - cb)
                        pa = psS.tile([128, 1024], f32, tag="big")
                        for n0 in range(0, wc, 512):
                            nn = min(512, wc - n0)
                            nc.tensor.matmul(
                                pa[:, n0 : n0 + nn],
                                qT[h // 2][rbase : rbase + 64, t * 128 : t * 128 + 128],
                                kT[h // 2][rbase : rbase + 64, cb + n0 : cb + n0 + nn],
                                start=True,
                                stop=True,
                                tile_position=(rbase, 0),
                            )
                        # causal mask on the diagonal block (always in last chunk)
                        if cb <= t * 128 < cb + wc:
                            off = t * 128 - cb
                            nc.vector.tensor_add(
                                pa[:, off : off + 128], pa[:, off : off + 128], tri[:]
                            )
                        nc.scalar.activation(
                            expA[:, cb : cb + wc],
                            pa[:, 0:wc],
                            Exp,
                            bias=0.0,
                            scale=SC,
                            accum_out=lsum[:, cidx : cidx + 1],
                        )
                    if nchunks == 2:
                        ltot = sbr.tile([128, 1], f32, tag="ltot")
                        nc.vector.tensor_add(ltot[:], lsum[:, 0:1], lsum[:, 1:2])
                        nc.vector.reciprocal(recip_all[:, t : t + 1], ltot[:])
                    else:
                        nc.vector.reciprocal(recip_all[:, t : t + 1], lsum[:, 0:1])
                    nc.vector.tensor_scalar_mul(
                        expA[:, 0:W], expA[:, 0:W], recip_all[:, t : t + 1]
                    )
                    nc.sync.dma_start(
                        attn_ap[h, t * 128 : t * 128 + 128, 0:W], expA[:, 0:W]
                    )

            # ----- recip rows for this pair -----
            rrows = []
            for par in range(2):
                prt = psS.tile([128, 1024], f32, tag="big")
                nc.tensor.transpose(prt[0:NT, 0:128], recips[par][:], ident[:])
                ri16 = sbr.tile([16, 128], f32r, tag="ri16")
                nc.vector.tensor_copy(ri16[:], prt[0:NT, 0:128])
                rd = dsc.tile([16, 128], f32r)
                nc.sync.dma_start(rd[:], ri16[:])
                rrow = sbr.tile([1, S], f32r, tag="rrow", name=f"rrow_{par}")
                nc.sync.dma_start(rrow[:], rd[:].rearrange("a b -> () (a b)"))
                rrows.append(rrow)

            # ----- pass B: sT, exp, ctx (1024-wide i-chunks) -----
            BCW = 1024
            for ic in range(S // BCW):
                i0 = ic * BCW
                jbmax = 8 * ic + 8
                pctxs = []
                rbs = []
                for par in range(2):
                    prb = psS.tile([128, 1024], f32, tag="big")
                    for half in range(2):
                        nc.tensor.matmul(
                            prb[0:64, half * 512 : half * 512 + 512],
                            ones[0:1, 0:64],
                            rrows[par][:, i0 + half * 512 : i0 + half * 512 + 512],
                            start=True,
                            stop=True,
                        )
                    rb = sbw.tile([64, 1024], f32, tag="rb", name=f"rb_{par}")
                    nc.vector.tensor_copy(rb[:], prb[0:64, :])
                    rbs.append(rb)
                    pctxs.append(psC.tile([64, 1024], f32, tag="ctx", name=f"pctx_{par}"))
                for jb in range(jbmax):
                    # columns i < jb*128 are fully masked; only compute
                    # the active tail [off, BCW) of this i-chunk
                    off = max(0, jb * 128 - i0)
                    na = BCW - off
                    for par in range(2):
                        h = 2 * g + par
                        rbase = par * 64
                        segs = [
                            (s0_, min(s0_ + 512, BCW))
                            for s0_ in range(0, BCW, 512)
                            if s0_ + 512 > off
                        ]
                        pB = psS.tile([128, 1024], f32, tag="big")
                        for s0_, s1_ in segs:
                            a0 = max(s0_, off)
                            nc.tensor.matmul(
                                pB[:, a0:s1_],
                                kT[h // 2][rbase : rbase + 64, jb * 128 : jb * 128 + 128],
                                qT[h // 2][rbase : rbase + 64, i0 + a0 : i0 + s1_],
                                start=True,
                                stop=True,
                                tile_position=(rbase, 0),
                            )
                        if jb >= 8 * ic:
                            nc.vector.tensor_add(
                                pB[:, off : off + 128], pB[:, off : off + 128], trit[:]
                            )
                        eB = sbw.tile([128, 1024], bf16, tag="expB")
                        nc.scalar.activation(
                            eB[:, off:BCW], pB[:, off:BCW], Exp, bias=0.0, scale=SC
                        )
                        for s0_, s1_ in segs:
                            a0 = max(s0_, off)
                            last_jb_seg = (
                                8 * ic + 3 if s1_ - i0 % BCW <= 512 and s1_ == 512 else jbmax - 1
                            )
                            nc.tensor.matmul(
                                pctxs[par][:, a0:s1_],
                                vna[:, jb, h * 64 : h * 64 + 64],
                                eB[:, a0:s1_],
                                start=(jb == 0),
                                stop=(jb == last_jb_seg),
                            )
                for par in range(2):
                    h = 2 * g + par
                    nc.vector.tensor_mul(
                        ctxT[h][:, i0 : i0 + BCW], pctxs[par][:], rbs[par][:]
                    )

        # ---- output projection xT = woT.T @ ctxT ----
        xt_ap = d_xt.ap()
        for ot in range(4):
            for ic in range(NIC):
                px = psS.tile([128, 1024], f32, tag="big")
                for c4 in range(HPC):
                    nc.tensor.matmul(
                        px[:, 0:512],
                        wo[:, c4, ot * 128 : ot * 128 + 128],
                        ctxT[c4][:, ic * ICW : ic * ICW + ICW],
                        start=(c4 == 0),
                        stop=(c4 == HPC - 1),
                    )
                xs = sbw.tile([128, 512], f32, tag="xs")
                nc.vector.tensor_copy(xs[:], px[:, 0:512])
                nc.sync.dma_start(
                    xt_ap[ot * 128 : ot * 128 + 128, ic * ICW : ic * ICW + ICW], xs[:]
                )

    nc.compile()
    return nc


_NC_CACHE = []


def _get_nc():
    if not _NC_CACHE:
        _NC_CACHE.append(_build_nc())
    return _NC_CACHE[0]


def _numpy_fallback(Q, K, V, mask, w_q, b_q, w_k, b_k, w_v, b_v, w_o, b_o):
    def split_heads(x):
        return x.reshape(B, S, H, DH).transpose(0, 2, 1, 3)

    q = split_heads(Q @ w_q.T + b_q)
    k = split_heads(K @ w_k.T + b_k)
    v = split_heads(V @ w_v.T + b_v)
    scores = np.einsum("bhqd,bhkd->bhqk", q, k) / math.sqrt(DH)
    scores = np.where(mask, scores, -np.inf)
    scores -= scores.max(axis=-1, keepdims=True)
    np.exp(scores, out=scores)
    attention = scores / scores.sum(axis=-1, keepdims=True)
    ctx = np.einsum("bhqk,bhkd->bhqd", attention, v)
    x = ctx.transpose(0, 2, 1, 3).reshape(B, S, D) @ w_o.T + b_o
    return x.astype(np.float32), attention.astype(np.float32)


def kernel(**inputs):
    from concourse.bass_utils import run_bass_kernel_spmd

    Q = np.asarray(inputs["Q"], dtype=np.float32)
    K = np.asarray(inputs["K"], dtype=np.float32)
    V = np.asarray(inputs["V"], dtype=np.float32)
    mask = np.asarray(inputs["mask"])
    w_q = np.asarray(inputs["w_q"], dtype=np.float32)
    b_q = np.asarray(inputs["b_q"], dtype=np.float32)
    w_k = np.asarray(inputs["w_k"], dtype=np.float32)
    b_k = np.asarray(inputs["b_k"], dtype=np.float32)
    w_v = np.asarray(inputs["w_v"], dtype=np.float32)
    b_v = np.asarray(inputs["b_v"], dtype=np.float32)
    w_o = np.asarray(inputs["w_o"], dtype=np.float32)
    b_o = np.asarray(inputs["b_o"], dtype=np.float32)

    m2 = mask.reshape(S, S)
    if not np.array_equal(m2, np.tril(np.ones((S, S), dtype=bool))):
        return _numpy_fallback(
            Q, K, V, mask, w_q, b_q, w_k, b_k, w_v, b_v, w_o, b_o
        )

    tri = np.triu(np.full((128, 128), NEG, dtype=np.float32), 1)
    trit = np.ascontiguousarray(tri.T)
    ident = np.eye(128, dtype=np.float32)
    ones = np.ones((1, 128), dtype=np.float32)

    qts = [np.ascontiguousarray(Q[b].T) for b in range(B)]
    kts = [np.ascontiguousarray(K[b].T) for b in range(B)]
    vts = [np.ascontiguousarray(V[b].T) for b in range(B)]

    in_maps = []
    for c in range(8):
        b, hg = c // 2, c % 2
        hs = slice(hg * 256, hg * 256 + 256)
        in_maps.append(
            {
                "qt": qts[b],
                "kt": kts[b],
                "vt": vts[b],
                "wq": np.ascontiguousarray(w_q[hs, :].T),
                "wk": np.ascontiguousarray(w_k[hs, :].T),
                "wv": np.ascontiguousarray(w_v[hs, :].T),
                "wo": np.ascontiguousarray(w_o[:, hs].T.reshape(4, 64, 512).transpose(1, 0, 2)),
                "bq": np.ascontiguousarray(b_q[hs].reshape(256, 1)),
                "bk": np.ascontiguousarray(b_k[hs].reshape(256, 1)),
                "bv": np.ascontiguousarray(b_v[hs].reshape(1, 256)),
                "tri": tri,
                "trit": trit,
                "ident": ident,
                "ones": ones,
            }
        )

    nc = _get_nc()
    res = run_bass_kernel_spmd(nc, in_maps, core_ids=list(range(8)))
    global LAST_RESULT
    LAST_RESULT = res

    attention = np.empty((B, H, S, S), dtype=np.float32)
    x = np.empty((B, S, D), dtype=np.float32)
    for b in range(B):
        r0 = res.results[2 * b]
        r1 = res.results[2 * b + 1]
        attention[b, 0:4] = r0["attn"]
        attention[b, 4:8] = r1["attn"]
        x[b] = (r0["xt"] + r1["xt"]).T + b_o
    return x, attention
